# revision 22
# baseline (speedup 1.0000x reference)
"""Trainium2 Bass kernel for CausalTrajectoryPrediction (batched per-branch MLPs).

Math (per branch n of N=512, H=256, M=64):
    x_off = x with element n zeroed ; x_diag = only element n kept
    h1 = relu(W1a[n] @ x_off)            # [H]
    r1 = relu(W1b[n] @ h1)               # [M]
    r2 = relu(W2b[n] @ relu(W2a[n] @ x_diag + b2a[n]) + b2b[n])   # [1]
    h4 = relu(W4a[n] @ [r1; r2] + b4a[n])                          # [H]
    pred = relu(W4b[n] @ h4 + b4b[n])                              # [2]
    out[n] = pred[0] + noise[n] * pred[1]

Device strategy (8 cores, 64 branches each, expert-parallel):
  * W1a[n] @ x_off == W1a[n] @ x - W1a[n,:,n]*x[n]; the shared matvec is done
    on-device (weights stationary, x moving), the diagonal correction is a
    tiny host-side gather shipped as a [128,128] f32 tile (computed from the
    bf16-rounded operands so the subtraction cancels exactly).
  * The self-excite path (r2) only touches W2a's diagonal -> computed on host
    (512 branches x ~8 flops) and folded into an effective stage-4 bias:
    bias4_eff = b4a + W4a[:,:,64] * r2.  Stage 4 then contracts over m=0..63.
  * Weights + moving activations in bf16 (PSUM accumulation f32, all bias /
    correction math f32): f32 matmuls lower to 2x LDWEIGHTS+MATMUL passes on
    TRN2, and bf16 stationaries get fast-weight-load; bf16 also halves the
    32MB weight stream.
  * All weights are pre-transposed on host into [K-partition, free] layouts so
    each per-branch matvec is a single matmul with the activation vector as
    the moving operand; outputs land on PSUM partitions in exactly the layout
    the next stage consumes.
  * DMAs all issue on the sync (HWDGE/SP) ring -> FIFO in program order:
    the W1a stream first (paces stage-1 PE), then stage-2/4 weights arrive
    just-in-time for the tail stages.
"""

import ml_dtypes
import numpy as np

import concourse.bacc as bacc
import concourse.bass as bass
import concourse.mybir as mybir
import concourse.tile as tile
from concourse.bass_utils import run_bass_kernel_spmd

F32 = mybir.dt.float32
BF16 = mybir.dt.float16
NP_BF16 = np.float16
NCORES = 8
N, H, M = 512, 256, 64
J = N // NCORES  # 64 branches per core

_CACHE = {}


def _build_nc():
    if "nc" in _CACHE:
        return _CACHE["nc"]

    nc = bacc.Bacc(
        "TRN2", target_bir_lowering=False, debug=False, enable_asserts=False,
        num_devices=NCORES,
    )

    # --- DRAM I/O (per-core shapes) ---
    # w1t[i, j*256+h]              = W1a[g, h, i]           (g = 64*core + j)
    # w2t[hl, j*128+hc*64+m]       = W1b[g, m, hc*128+hl]
    # w4t[64*(j%2)+m, (j//2)*256+h]= W4a[g, h, m]   (m<64; col 64 folded in bias)
    # w5t[hl, j*4+hc*2+o]          = W4b[g, o, hc*128+hl]
    # xbf[p, ic] = x[128*ic+p]                     (bf16 moving operand)
    # aux1 = [corr(128) | bias4(128)]   -> [128, 256] f32
    #   corr[p, 2j+hh]  = bf16(W1a[g, hh*128+p, g]) * bf16(x[g])
    #   bias4[p, 2j+hh] = b4a[g, hh*128+p] + W4a[g, hh*128+p, 64]*r2_host[g]
    # aux2 = [b4bt(64) | noise2(64) | ones(1)]     -> [2, 129] f32
    w1t = nc.dram_tensor("w1t", [512, J * H], BF16, kind="ExternalInput").ap()
    w2t = nc.dram_tensor("w2t", [128, J * 2 * 64], BF16, kind="ExternalInput").ap()
    w4t = nc.dram_tensor("w4t", [128, (J // 2) * H], BF16, kind="ExternalInput").ap()
    w5t = nc.dram_tensor("w5t", [128, J * 4], BF16, kind="ExternalInput").ap()
    xbf = nc.dram_tensor("xbf", [128, 4], BF16, kind="ExternalInput").ap()
    aux1 = nc.dram_tensor("aux1", [128, 256], F32, kind="ExternalInput").ap()
    # aux3 = [dmask(32) | msk2(64) | b4bp(2)] -> [64, 98] f32
    #   dmask[2jl+o, jf] = (jl==jf)
    #   msk2[2jl+o, 32g+jf] = (jl==jf) * (1 if o==0 else noise[64c+32g+jf])
    #   b4bp[2jl+o, g] = b4b[64c+32g+jl, o]
    aux3 = nc.dram_tensor("aux3", [64, 98], F32, kind="ExternalInput").ap()
    out = nc.dram_tensor("out", [1, J], F32, kind="ExternalOutput").ap()

    # stage-1 free-dim chunk plan: small first chunk (fast rampup), 1 MiB rest
    CHUNK_COLS = [1024, 3072, 4096, 4096, 4096]

    with tile.TileContext(nc) as tc:
        with (
            tc.tile_pool(name="stream", bufs=16) as sp,
            tc.tile_pool(name="res", bufs=1) as rp,
            tc.tile_pool(name="psum", bufs=1, space=bass.MemorySpace.PSUM) as pp,
        ):
            # small resident tensors on the scalar (ACT) HWDGE ring first
            xbf_t = rp.tile([128, 4], BF16)
            nc.scalar.dma_start(xbf_t[:, :], xbf)
            aux1_t = rp.tile([128, 256], F32)
            aux3_t = rp.tile([64, 98], F32)
            w5s = rp.tile([128, J * 4], BF16)

            def aux_dma():
                nc.scalar.dma_start(aux1_t[:, :], aux1)
                nc.scalar.dma_start(aux3_t[:, :], aux3)
                nc.scalar.dma_start(w5s[:, :], w5t)

            corrt = aux1_t[:, 0:128]
            bias4t = aux1_t[:, 128:256]
            dmask = aux3_t[:, 0:32]
            msk2 = aux3_t[:, 32:96]
            b4bp = aux3_t[:, 96:98]

            w2s = rp.tile([128, J * 2 * 64], BF16)
            w4s = rp.tile([128, (J // 2) * H], BF16)

            y1_a = pp.tile([128, 64], F32)
            y1_b = pp.tile([128, 64], F32)
            y1h = [y1_a, y1_b]
            zsrc = rp.tile([128, 128], BF16)
            nc.vector.memset(zsrc[:, :], 0.0)
            h1f = rp.tile([128, 128], F32)
            h1sb = rp.tile([128, 128], BF16)
            psum2 = pp.tile([128, J], F32)
            nc.vector.memset(psum2[:, :], 0.0)
            r1cols = rp.tile([128, J], BF16)
            psum4 = pp.tile([128, 128], F32)
            h4f = rp.tile([128, 128], F32)
            h4cols = rp.tile([128, 128], BF16)
            psum5g = pp.tile([64, J], F32)
            ttr_dump = rp.tile([64, J], F32)
            predcol = rp.tile([64, 2], F32)

            def s1_groups(chunk_cols, col0, half):
                """stage-1 matmuls for w1t cols [col0, ...); dual-ring stream.
                Consumed ic-outer (chunk-arrival order); y1 bank pre-cleared by
                a zero matmul so every pass accumulates with start=False."""
                nc.tensor.matmul(
                    y1h[half][:, 0:64], zsrc[:, 0:128], zsrc[:, 0:64],
                    start=True, stop=False, skip_group_check=True,
                )
                for gidx, ncols in enumerate(chunk_cols):
                    tiles_u = []
                    for ic in range(4):
                        wt = sp.tile([128, ncols], BF16, tag="w1s")
                        eng = nc.sync if ic % 2 == 0 else nc.scalar
                        eng.dma_start(
                            wt[:, :],
                            w1t[128 * ic : 128 * (ic + 1), col0 : col0 + ncols],
                        )
                        tiles_u.append(wt)
                    for ic in range(4):
                        for tt in range(ncols // 128):
                            t = col0 // 128 + tt
                            nc.tensor.matmul(
                                y1h[half][:, t % 64 : t % 64 + 1],
                                tiles_u[ic][:, 128 * tt : 128 * (tt + 1)],
                                xbf_t[:, ic : ic + 1],
                                start=False,
                                stop=(ic == 3 and gidx == len(chunk_cols) - 1),
                                skip_group_check=True,
                            )
                    col0 += ncols

            def tail_weights_dma(half):
                lo = 4096 * half
                nc.sync.dma_start(w2s[:, lo : lo + 2048], w2t[:, lo : lo + 2048])
                nc.scalar.dma_start(
                    w2s[:, lo + 2048 : lo + 4096], w2t[:, lo + 2048 : lo + 4096]
                )
                nc.scalar.dma_start(w4s[:, lo : lo + 2048], w4t[:, lo : lo + 2048])
                nc.sync.dma_start(
                    w4s[:, lo + 2048 : lo + 4096], w4t[:, lo + 2048 : lo + 4096]
                )

            def tail_half(half):
                """stages 2/4/5 for branches [32*half, 32*half+32)"""
                lo = 64 * half  # h1 col offset
                jlo = 32 * half
                # h1 = relu(y1 - corr)
                nc.vector.tensor_sub(
                    h1f[:, lo : lo + 64], y1h[half][:, 0:64], corrt[:, lo : lo + 64]
                )
                nc.vector.tensor_scalar_max(
                    h1sb[:, lo : lo + 64], h1f[:, lo : lo + 64], 0.0
                )
                # stage 2
                for j in range(jlo, jlo + 32):
                    off = 64 * (j % 2)
                    for hc in range(2):
                        nc.tensor.matmul(
                            psum2[off : off + 64, j : j + 1],
                            w2s[:, j * 128 + hc * 64 : j * 128 + hc * 64 + 64],
                            h1sb[:, 2 * j + hc : 2 * j + hc + 1],
                            start=(hc == 0),
                            stop=(hc == 1),
                        )
                nc.vector.tensor_scalar_max(
                    r1cols[:, jlo : jlo + 32], psum2[:, jlo : jlo + 32], 0.0
                )
                # stage 4 (pair-shared K=128 stationaries)
                for u_ in range(16 * half, 16 * half + 16):
                    for hc in range(2):
                        for par in range(2):
                            j = 2 * u_ + par
                            nc.tensor.matmul(
                                psum4[:, hc * 64 + j : hc * 64 + j + 1],
                                w4s[:, u_ * 256 + hc * 128 : u_ * 256 + hc * 128 + 128],
                                r1cols[:, j : j + 1],
                                start=True,
                                stop=True,
                            )
                for hc in range(2):
                    c = hc * 64 + jlo
                    nc.vector.tensor_add(
                        h4f[:, c : c + 32], psum4[:, c : c + 32], bias4t[:, c : c + 32]
                    )
                    nc.vector.tensor_scalar_max(
                        h4cols[:, c : c + 32], h4f[:, c : c + 32], 0.0
                    )
                # stage 5 (32-branch packed stationary, F=32)
                grp = half
                for hc in range(2):
                    nc.tensor.matmul(
                        psum5g[0:64, 32 * grp : 32 * grp + 32],
                        w5s[:, 128 * grp + 64 * hc : 128 * grp + 64 * hc + 64],
                        h4cols[:, 64 * hc + 32 * grp : 64 * hc + 32 * grp + 32],
                        start=(hc == 0),
                        stop=(hc == 1),
                    )
                nc.vector.tensor_mul(
                    ttr_dump[:, 32 * grp : 32 * grp + 32],
                    psum5g[0:64, 32 * grp : 32 * grp + 32],
                    dmask,
                )
                nc.vector.tensor_reduce(
                    predcol[:, grp : grp + 1],
                    ttr_dump[:, 32 * grp : 32 * grp + 32],
                    mybir.AxisListType.X,
                    mybir.AluOpType.add,
                )

            # ---- schedule: half A's tail stages run while half B streams ----
            s1_groups([4096] * 2, 0, 0)
            aux_dma()
            tail_weights_dma(0)
            tail_half(0)
            s1_groups([4096] * 2, 8192, 1)
            tail_weights_dma(1)
            tail_half(1)

            # pred = relu(predcol + b4bp); out = msk2-combine (noise folded in)
            nc.vector.tensor_add(predcol[:, :], predcol[:, :], b4bp)
            nc.vector.tensor_scalar_max(predcol[:, :], predcol[:, :], 0.0)
            psum6 = pp.tile([1, J], F32)
            for grp in range(2):
                nc.tensor.matmul(
                    psum6[0:1, 32 * grp : 32 * grp + 32],
                    predcol[:, grp : grp + 1],
                    msk2[:, 32 * grp : 32 * grp + 32],
                    start=True,
                    stop=True,
                )
            yrow = rp.tile([1, J], F32)
            nc.vector.tensor_copy(yrow[0:1, :], psum6[0:1, :])
            nc.sync.dma_start(out, yrow[0:1, :])

    nc.compile()
    _CACHE["nc"] = nc
    return nc


def _bf(a):
    return np.ascontiguousarray(a.astype(NP_BF16))


def prep_core_inputs(inputs, c):
    """Host-side shard + layout prep for core c. inputs are np float32 arrays."""
    x = inputs["x"][0]  # [512]
    gi = np.arange(J * c, J * (c + 1))
    xg = x[gi]
    jj = np.arange(J)

    W1a_c = inputs["W1a"][gi]  # [64, 256, 512]
    w1t = _bf(W1a_c.transpose(2, 0, 1).reshape(512, J * H))

    # self-excite path entirely on host (tiny), folded into stage-4 bias
    dW2 = inputs["W2a"][gi, :, gi]  # [64, 2]
    h2 = np.maximum(dW2 * xg[:, None] + inputs["b2a"][gi], 0.0)
    r2 = np.maximum((inputs["W2b"][gi, 0] * h2).sum(-1) + inputs["b2b"][gi, 0], 0.0)

    # correction computed from the bf16-rounded operands (exact cancellation
    # of the diagonal term the device's bf16 matmul actually added)
    dW1 = W1a_c[jj, :, gi].astype(NP_BF16).astype(np.float32)  # [64, 256]
    xg_bf = xg.astype(NP_BF16).astype(np.float32)
    corr_jh = dW1 * xg_bf[:, None]
    corrt = corr_jh.reshape(J, 2, 128).transpose(2, 0, 1).reshape(128, 128)

    W4a_c = inputs["W4a"][gi]  # [64, 256, 65]
    bias4_jh = inputs["b4a"][gi] + W4a_c[:, :, 64] * r2[:, None]
    # h4 col layout = hc*64 + j
    bias4t = bias4_jh.reshape(J, 2, 128).transpose(2, 1, 0).reshape(128, 128)

    aux1 = np.ascontiguousarray(
        np.concatenate([corrt, bias4t], axis=1), dtype=np.float32
    )
    xbf = _bf(x.reshape(4, 128).T)  # [128, 4]

    # aux3: dmask | msk2 | b4bp   (stage-5 diag-extract + final combine)
    noise_c = inputs["noise"][gi]
    dmask = np.repeat(np.eye(32, dtype=np.float32), 2, axis=0)  # [64, 32]
    msk2 = np.zeros((64, 64), np.float32)
    for grp in range(2):
        jf = np.arange(32)
        msk2[2 * jf, 32 * grp + jf] = 1.0
        msk2[2 * jf + 1, 32 * grp + jf] = noise_c[32 * grp + jf]
    b4bp = (
        inputs["b4b"][gi].reshape(2, 32, 2).transpose(1, 2, 0).reshape(64, 2)
    )  # [2jl+o, grp]
    aux3 = np.ascontiguousarray(
        np.concatenate([dmask, msk2, b4bp], axis=1), dtype=np.float32
    )

    W1b_c = inputs["W1b"][gi]  # [64, 64, 256]
    w2t = _bf(
        W1b_c.reshape(J, 64, 2, 128).transpose(3, 0, 2, 1).reshape(128, J * 2 * 64)
    )

    # stage 4: branch pairs stacked on partitions (even j -> rows 0..63)
    W4m = W4a_c[:, :, 0:64]  # [j, h, m]
    T4 = W4m.reshape(J // 2, 2, H, 64)  # [u, par, h, m]
    w4t = _bf(T4.transpose(1, 3, 0, 2).reshape(128, (J // 2) * H))

    # w5p[hl, grp*128 + hc*64 + 2jl+o] = W4b[g(32grp+jl), o, hc*128+hl]
    W4b_c = inputs["W4b"][gi]  # [64, 2, 256]
    w5t = _bf(
        W4b_c.reshape(2, 32, 2, 2, 128).transpose(4, 0, 3, 1, 2).reshape(128, J * 4)
    )

    return {
        "w1t": w1t, "w2t": w2t, "w4t": w4t, "w5t": w5t,
        "xbf": xbf, "aux1": aux1, "aux3": aux3,
    }


def run(inputs, trace=False, **kw):
    inputs = {k: np.asarray(v, dtype=np.float32) for k, v in inputs.items()}
    nc = _build_nc()
    in_maps = [prep_core_inputs(inputs, c) for c in range(NCORES)]
    res = run_bass_kernel_spmd(
        nc, in_maps, core_ids=list(range(NCORES)), trace=trace, **kw
    )
    out = np.concatenate([res.results[c]["out"] for c in range(NCORES)], axis=1)
    return out.astype(np.float32), res


def kernel(**inputs):
    out, _ = run(inputs)
    return out


# revision 23
# speedup vs baseline: 1.0205x; 1.0205x over previous
"""Trainium2 Bass kernel for CausalTrajectoryPrediction (batched per-branch MLPs).

Math (per branch n of N=512, H=256, M=64):
    x_off = x with element n zeroed ; x_diag = only element n kept
    h1 = relu(W1a[n] @ x_off)            # [H]
    r1 = relu(W1b[n] @ h1)               # [M]
    r2 = relu(W2b[n] @ relu(W2a[n] @ x_diag + b2a[n]) + b2b[n])   # [1]
    h4 = relu(W4a[n] @ [r1; r2] + b4a[n])                          # [H]
    pred = relu(W4b[n] @ h4 + b4b[n])                              # [2]
    out[n] = pred[0] + noise[n] * pred[1]

Device strategy (8 cores, 64 branches each, expert-parallel):
  * W1a[n] @ x_off == W1a[n] @ x - W1a[n,:,n]*x[n]; the shared matvec is done
    on-device (weights stationary, x moving), the diagonal correction is a
    tiny host-side gather shipped as a [128,128] f32 tile (computed from the
    bf16-rounded operands so the subtraction cancels exactly).
  * The self-excite path (r2) only touches W2a's diagonal -> computed on host
    (512 branches x ~8 flops) and folded into an effective stage-4 bias:
    bias4_eff = b4a + W4a[:,:,64] * r2.  Stage 4 then contracts over m=0..63.
  * Weights + moving activations in bf16 (PSUM accumulation f32, all bias /
    correction math f32): f32 matmuls lower to 2x LDWEIGHTS+MATMUL passes on
    TRN2, and bf16 stationaries get fast-weight-load; bf16 also halves the
    32MB weight stream.
  * All weights are pre-transposed on host into [K-partition, free] layouts so
    each per-branch matvec is a single matmul with the activation vector as
    the moving operand; outputs land on PSUM partitions in exactly the layout
    the next stage consumes.
  * DMAs all issue on the sync (HWDGE/SP) ring -> FIFO in program order:
    the W1a stream first (paces stage-1 PE), then stage-2/4 weights arrive
    just-in-time for the tail stages.
"""

import ml_dtypes
import numpy as np

import concourse.bacc as bacc
import concourse.bass as bass
import concourse.mybir as mybir
import concourse.tile as tile
from concourse.bass_utils import run_bass_kernel_spmd

F32 = mybir.dt.float32
BF16 = mybir.dt.float16
NP_BF16 = np.float16
NCORES = 8
N, H, M = 512, 256, 64
J = N // NCORES  # 64 branches per core

_CACHE = {}


def _build_nc():
    if "nc" in _CACHE:
        return _CACHE["nc"]

    nc = bacc.Bacc(
        "TRN2", target_bir_lowering=False, debug=False, enable_asserts=False,
        num_devices=NCORES,
    )

    # --- DRAM I/O (per-core shapes) ---
    # w1t[i, j*256+h]              = W1a[g, h, i]           (g = 64*core + j)
    # w2t[hl, j*128+hc*64+m]       = W1b[g, m, hc*128+hl]
    # w4t[64*(j%2)+m, (j//2)*256+h]= W4a[g, h, m]   (m<64; col 64 folded in bias)
    # w5t[hl, j*4+hc*2+o]          = W4b[g, o, hc*128+hl]
    # xbf[p, ic] = x[128*ic+p]                     (bf16 moving operand)
    # aux1 = [corr(128) | bias4(128)]   -> [128, 256] f32
    #   corr[p, 2j+hh]  = bf16(W1a[g, hh*128+p, g]) * bf16(x[g])
    #   bias4[p, 2j+hh] = b4a[g, hh*128+p] + W4a[g, hh*128+p, 64]*r2_host[g]
    # aux2 = [b4bt(64) | noise2(64) | ones(1)]     -> [2, 129] f32
    w1t = nc.dram_tensor("w1t", [512, J * H], BF16, kind="ExternalInput").ap()
    w2t = nc.dram_tensor("w2t", [128, J * 2 * 64], BF16, kind="ExternalInput").ap()
    w4t = nc.dram_tensor("w4t", [128, (J // 2) * H], BF16, kind="ExternalInput").ap()
    w5t = nc.dram_tensor("w5t", [128, J * 4], BF16, kind="ExternalInput").ap()
    xbf = nc.dram_tensor("xbf", [128, 4], BF16, kind="ExternalInput").ap()
    aux1 = nc.dram_tensor("aux1", [128, 256], F32, kind="ExternalInput").ap()
    # aux3 = [dmask(32) | msk2(64) | b4bp(2)] -> [64, 98] f32
    #   dmask[2jl+o, jf] = (jl==jf)
    #   msk2[2jl+o, 32g+jf] = (jl==jf) * (1 if o==0 else noise[64c+32g+jf])
    #   b4bp[2jl+o, g] = b4b[64c+32g+jl, o]
    aux3 = nc.dram_tensor("aux3", [64, 98], F32, kind="ExternalInput").ap()
    out = nc.dram_tensor("out", [1, J], F32, kind="ExternalOutput").ap()

    # stage-1 free-dim chunk plan: small first chunk (fast rampup), 1 MiB rest
    CHUNK_COLS = [1024, 3072, 4096, 4096, 4096]

    with tile.TileContext(nc) as tc:
        with (
            tc.tile_pool(name="stream", bufs=24) as sp,
            tc.tile_pool(name="res", bufs=1) as rp,
            tc.tile_pool(name="psum", bufs=1, space=bass.MemorySpace.PSUM) as pp,
        ):
            # small resident tensors on the scalar (ACT) HWDGE ring first
            xbf_t = rp.tile([128, 4], BF16)
            nc.scalar.dma_start(xbf_t[:, :], xbf)
            aux1_t = rp.tile([128, 256], F32)
            aux3_t = rp.tile([64, 98], F32)
            w5s = rp.tile([128, J * 4], BF16)

            def aux_dma():
                nc.gpsimd.dma_start(aux1_t[:, :], aux1)
                nc.gpsimd.dma_start(aux3_t[:, :], aux3)
                nc.gpsimd.dma_start(w5s[:, :], w5t)

            corrt = aux1_t[:, 0:128]
            bias4t = aux1_t[:, 128:256]
            dmask = aux3_t[:, 0:32]
            msk2 = aux3_t[:, 32:96]
            b4bp = aux3_t[:, 96:98]

            w2s = rp.tile([128, J * 2 * 64], BF16)
            w4s = rp.tile([128, (J // 2) * H], BF16)

            y1_a = pp.tile([128, 64], F32)
            y1_b = pp.tile([128, 64], F32)
            y1h = [y1_a, y1_b]
            zsrc = rp.tile([128, 128], BF16)
            nc.vector.memset(zsrc[:, :], 0.0)
            h1f = rp.tile([128, 128], F32)
            h1sb = rp.tile([128, 128], BF16)
            psum2 = pp.tile([128, J], F32)
            nc.vector.memset(psum2[:, :], 0.0)
            r1cols = rp.tile([128, J], BF16)
            psum4 = pp.tile([128, 128], F32)
            h4f = rp.tile([128, 128], F32)
            h4cols = rp.tile([128, 128], BF16)
            psum5g = pp.tile([64, J], F32)
            ttr_dump = rp.tile([64, J], F32)
            predcol = rp.tile([64, 2], F32)

            def s1_groups(chunk_cols, col0, half):
                """stage-1 matmuls for w1t cols [col0, ...); dual-ring stream.
                Consumed ic-outer (chunk-arrival order); y1 bank pre-cleared by
                a zero matmul so every pass accumulates with start=False."""
                nc.tensor.matmul(
                    y1h[half][:, 0:64], zsrc[:, 0:128], zsrc[:, 0:64],
                    start=True, stop=False, skip_group_check=True,
                )
                for gidx, ncols in enumerate(chunk_cols):
                    tiles_u = []
                    for ic in range(4):
                        wt = sp.tile([128, ncols], BF16, tag="w1s")
                        eng = nc.sync if ic % 2 == 0 else nc.scalar
                        eng.dma_start(
                            wt[:, :],
                            w1t[128 * ic : 128 * (ic + 1), col0 : col0 + ncols],
                        )
                        tiles_u.append(wt)
                    for ic in range(4):
                        for tt in range(ncols // 128):
                            t = col0 // 128 + tt
                            nc.tensor.matmul(
                                y1h[half][:, t % 64 : t % 64 + 1],
                                tiles_u[ic][:, 128 * tt : 128 * (tt + 1)],
                                xbf_t[:, ic : ic + 1],
                                start=False,
                                stop=(ic == 3 and gidx == len(chunk_cols) - 1),
                                skip_group_check=True,
                            )
                    col0 += ncols

            def tail_weights_dma(half):
                lo = 4096 * half
                nc.gpsimd.dma_start(w2s[:, lo : lo + 4096], w2t[:, lo : lo + 4096])
                nc.gpsimd.dma_start(w4s[:, lo : lo + 4096], w4t[:, lo : lo + 4096])

            def tail_half(half):
                """stages 2/4/5 for branches [32*half, 32*half+32)"""
                lo = 64 * half  # h1 col offset
                jlo = 32 * half
                # h1 = relu(y1 - corr)
                nc.vector.tensor_sub(
                    h1f[:, lo : lo + 64], y1h[half][:, 0:64], corrt[:, lo : lo + 64]
                )
                nc.vector.tensor_scalar_max(
                    h1sb[:, lo : lo + 64], h1f[:, lo : lo + 64], 0.0
                )
                # stage 2
                for j in range(jlo, jlo + 32):
                    off = 64 * (j % 2)
                    for hc in range(2):
                        nc.tensor.matmul(
                            psum2[off : off + 64, j : j + 1],
                            w2s[:, j * 128 + hc * 64 : j * 128 + hc * 64 + 64],
                            h1sb[:, 2 * j + hc : 2 * j + hc + 1],
                            start=(hc == 0),
                            stop=(hc == 1),
                        )
                nc.vector.tensor_scalar_max(
                    r1cols[:, jlo : jlo + 32], psum2[:, jlo : jlo + 32], 0.0
                )
                # stage 4 (pair-shared K=128 stationaries)
                for u_ in range(16 * half, 16 * half + 16):
                    for hc in range(2):
                        for par in range(2):
                            j = 2 * u_ + par
                            nc.tensor.matmul(
                                psum4[:, hc * 64 + j : hc * 64 + j + 1],
                                w4s[:, u_ * 256 + hc * 128 : u_ * 256 + hc * 128 + 128],
                                r1cols[:, j : j + 1],
                                start=True,
                                stop=True,
                            )
                for hc in range(2):
                    c = hc * 64 + jlo
                    nc.vector.tensor_add(
                        h4f[:, c : c + 32], psum4[:, c : c + 32], bias4t[:, c : c + 32]
                    )
                    nc.vector.tensor_scalar_max(
                        h4cols[:, c : c + 32], h4f[:, c : c + 32], 0.0
                    )
                # stage 5 (32-branch packed stationary, F=32)
                grp = half
                for hc in range(2):
                    nc.tensor.matmul(
                        psum5g[0:64, 32 * grp : 32 * grp + 32],
                        w5s[:, 128 * grp + 64 * hc : 128 * grp + 64 * hc + 64],
                        h4cols[:, 64 * hc + 32 * grp : 64 * hc + 32 * grp + 32],
                        start=(hc == 0),
                        stop=(hc == 1),
                    )
                nc.vector.tensor_mul(
                    ttr_dump[:, 32 * grp : 32 * grp + 32],
                    psum5g[0:64, 32 * grp : 32 * grp + 32],
                    dmask,
                )
                nc.vector.tensor_reduce(
                    predcol[:, grp : grp + 1],
                    ttr_dump[:, 32 * grp : 32 * grp + 32],
                    mybir.AxisListType.X,
                    mybir.AluOpType.add,
                )

            # ---- schedule: half A's tail stages run while half B streams ----
            s1_groups([2048] * 4, 0, 0)
            aux_dma()
            tail_weights_dma(0)
            tail_weights_dma(1)
            tail_half(0)
            s1_groups([2048] * 4, 8192, 1)
            tail_half(1)

            # pred = relu(predcol + b4bp); out = msk2-combine (noise folded in)
            nc.vector.tensor_add(predcol[:, :], predcol[:, :], b4bp)
            nc.vector.tensor_scalar_max(predcol[:, :], predcol[:, :], 0.0)
            psum6 = pp.tile([1, J], F32)
            for grp in range(2):
                nc.tensor.matmul(
                    psum6[0:1, 32 * grp : 32 * grp + 32],
                    predcol[:, grp : grp + 1],
                    msk2[:, 32 * grp : 32 * grp + 32],
                    start=True,
                    stop=True,
                )
            yrow = rp.tile([1, J], F32)
            nc.vector.tensor_copy(yrow[0:1, :], psum6[0:1, :])
            nc.sync.dma_start(out, yrow[0:1, :])

    nc.compile()
    _CACHE["nc"] = nc
    return nc


def _bf(a):
    return np.ascontiguousarray(a.astype(NP_BF16))


def prep_core_inputs(inputs, c):
    """Host-side shard + layout prep for core c. inputs are np float32 arrays."""
    x = inputs["x"][0]  # [512]
    gi = np.arange(J * c, J * (c + 1))
    xg = x[gi]
    jj = np.arange(J)

    W1a_c = inputs["W1a"][gi]  # [64, 256, 512]
    w1t = _bf(W1a_c.transpose(2, 0, 1).reshape(512, J * H))

    # self-excite path entirely on host (tiny), folded into stage-4 bias
    dW2 = inputs["W2a"][gi, :, gi]  # [64, 2]
    h2 = np.maximum(dW2 * xg[:, None] + inputs["b2a"][gi], 0.0)
    r2 = np.maximum((inputs["W2b"][gi, 0] * h2).sum(-1) + inputs["b2b"][gi, 0], 0.0)

    # correction computed from the bf16-rounded operands (exact cancellation
    # of the diagonal term the device's bf16 matmul actually added)
    dW1 = W1a_c[jj, :, gi].astype(NP_BF16).astype(np.float32)  # [64, 256]
    xg_bf = xg.astype(NP_BF16).astype(np.float32)
    corr_jh = dW1 * xg_bf[:, None]
    corrt = corr_jh.reshape(J, 2, 128).transpose(2, 0, 1).reshape(128, 128)

    W4a_c = inputs["W4a"][gi]  # [64, 256, 65]
    bias4_jh = inputs["b4a"][gi] + W4a_c[:, :, 64] * r2[:, None]
    # h4 col layout = hc*64 + j
    bias4t = bias4_jh.reshape(J, 2, 128).transpose(2, 1, 0).reshape(128, 128)

    aux1 = np.ascontiguousarray(
        np.concatenate([corrt, bias4t], axis=1), dtype=np.float32
    )
    xbf = _bf(x.reshape(4, 128).T)  # [128, 4]

    # aux3: dmask | msk2 | b4bp   (stage-5 diag-extract + final combine)
    noise_c = inputs["noise"][gi]
    dmask = np.repeat(np.eye(32, dtype=np.float32), 2, axis=0)  # [64, 32]
    msk2 = np.zeros((64, 64), np.float32)
    for grp in range(2):
        jf = np.arange(32)
        msk2[2 * jf, 32 * grp + jf] = 1.0
        msk2[2 * jf + 1, 32 * grp + jf] = noise_c[32 * grp + jf]
    b4bp = (
        inputs["b4b"][gi].reshape(2, 32, 2).transpose(1, 2, 0).reshape(64, 2)
    )  # [2jl+o, grp]
    aux3 = np.ascontiguousarray(
        np.concatenate([dmask, msk2, b4bp], axis=1), dtype=np.float32
    )

    W1b_c = inputs["W1b"][gi]  # [64, 64, 256]
    w2t = _bf(
        W1b_c.reshape(J, 64, 2, 128).transpose(3, 0, 2, 1).reshape(128, J * 2 * 64)
    )

    # stage 4: branch pairs stacked on partitions (even j -> rows 0..63)
    W4m = W4a_c[:, :, 0:64]  # [j, h, m]
    T4 = W4m.reshape(J // 2, 2, H, 64)  # [u, par, h, m]
    w4t = _bf(T4.transpose(1, 3, 0, 2).reshape(128, (J // 2) * H))

    # w5p[hl, grp*128 + hc*64 + 2jl+o] = W4b[g(32grp+jl), o, hc*128+hl]
    W4b_c = inputs["W4b"][gi]  # [64, 2, 256]
    w5t = _bf(
        W4b_c.reshape(2, 32, 2, 2, 128).transpose(4, 0, 3, 1, 2).reshape(128, J * 4)
    )

    return {
        "w1t": w1t, "w2t": w2t, "w4t": w4t, "w5t": w5t,
        "xbf": xbf, "aux1": aux1, "aux3": aux3,
    }


def run(inputs, trace=False, **kw):
    inputs = {k: np.asarray(v, dtype=np.float32) for k, v in inputs.items()}
    nc = _build_nc()
    in_maps = [prep_core_inputs(inputs, c) for c in range(NCORES)]
    res = run_bass_kernel_spmd(
        nc, in_maps, core_ids=list(range(NCORES)), trace=trace, **kw
    )
    out = np.concatenate([res.results[c]["out"] for c in range(NCORES)], axis=1)
    return out.astype(np.float32), res


def kernel(**inputs):
    out, _ = run(inputs)
    return out


# revision 24
# speedup vs baseline: 1.1001x; 1.0780x over previous
"""Trainium2 Bass kernel for CausalTrajectoryPrediction (batched per-branch MLPs).

Math (per branch n of N=512, H=256, M=64):
    x_off = x with element n zeroed ; x_diag = only element n kept
    h1 = relu(W1a[n] @ x_off)            # [H]
    r1 = relu(W1b[n] @ h1)               # [M]
    r2 = relu(W2b[n] @ relu(W2a[n] @ x_diag + b2a[n]) + b2b[n])   # [1]
    h4 = relu(W4a[n] @ [r1; r2] + b4a[n])                          # [H]
    pred = relu(W4b[n] @ h4 + b4b[n])                              # [2]
    out[n] = pred[0] + noise[n] * pred[1]

Device strategy (8 cores, 64 branches each, expert-parallel):
  * W1a[n] @ x_off == W1a[n] @ x - W1a[n,:,n]*x[n]; the shared matvec is done
    on-device (weights stationary, x moving), the diagonal correction is a
    tiny host-side gather shipped as a [128,128] f32 tile (computed from the
    bf16-rounded operands so the subtraction cancels exactly).
  * The self-excite path (r2) only touches W2a's diagonal -> computed on host
    (512 branches x ~8 flops) and folded into an effective stage-4 bias:
    bias4_eff = b4a + W4a[:,:,64] * r2.  Stage 4 then contracts over m=0..63.
  * Weights + moving activations in bf16 (PSUM accumulation f32, all bias /
    correction math f32): f32 matmuls lower to 2x LDWEIGHTS+MATMUL passes on
    TRN2, and bf16 stationaries get fast-weight-load; bf16 also halves the
    32MB weight stream.
  * All weights are pre-transposed on host into [K-partition, free] layouts so
    each per-branch matvec is a single matmul with the activation vector as
    the moving operand; outputs land on PSUM partitions in exactly the layout
    the next stage consumes.
  * DMAs all issue on the sync (HWDGE/SP) ring -> FIFO in program order:
    the W1a stream first (paces stage-1 PE), then stage-2/4 weights arrive
    just-in-time for the tail stages.
"""

import ml_dtypes
import numpy as np

import concourse.bacc as bacc
import concourse.bass as bass
import concourse.mybir as mybir
import concourse.tile as tile
from concourse.bass_utils import run_bass_kernel_spmd

F32 = mybir.dt.float32
BF16 = mybir.dt.float16
NP_BF16 = np.float16
NCORES = 8
N, H, M = 512, 256, 64
J = N // NCORES  # 64 branches per core

_CACHE = {}


def _build_nc():
    if "nc" in _CACHE:
        return _CACHE["nc"]

    nc = bacc.Bacc(
        "TRN2", target_bir_lowering=False, debug=False, enable_asserts=False,
        num_devices=NCORES,
    )

    # --- DRAM I/O (per-core shapes) ---
    # w1t[i, j*256+h]              = W1a[g, h, i]           (g = 64*core + j)
    # w2t[hl, j*128+hc*64+m]       = W1b[g, m, hc*128+hl]
    # w4t[64*(j%2)+m, (j//2)*256+h]= W4a[g, h, m]   (m<64; col 64 folded in bias)
    # w5t[hl, j*4+hc*2+o]          = W4b[g, o, hc*128+hl]
    # xbf[p, ic] = x[128*ic+p]                     (bf16 moving operand)
    # aux1 = [corr(128) | bias4(128)]   -> [128, 256] f32
    #   corr[p, 2j+hh]  = bf16(W1a[g, hh*128+p, g]) * bf16(x[g])
    #   bias4[p, 2j+hh] = b4a[g, hh*128+p] + W4a[g, hh*128+p, 64]*r2_host[g]
    # aux2 = [b4bt(64) | noise2(64) | ones(1)]     -> [2, 129] f32
    w1t = nc.dram_tensor("w1t", [512, J * H], BF16, kind="ExternalInput").ap()
    w2t = nc.dram_tensor("w2t", [128, J * 2 * 64], BF16, kind="ExternalInput").ap()
    w4t = nc.dram_tensor("w4t", [128, (J // 2) * H], BF16, kind="ExternalInput").ap()
    w5t = nc.dram_tensor("w5t", [128, J * 4], BF16, kind="ExternalInput").ap()
    xbf = nc.dram_tensor("xbf", [128, 4], BF16, kind="ExternalInput").ap()
    aux1 = nc.dram_tensor("aux1", [128, 256], F32, kind="ExternalInput").ap()
    # aux3 = [dmask(32) | msk2(64) | b4bp(2)] -> [64, 98] f32
    #   dmask[2jl+o, jf] = (jl==jf)
    #   msk2[2jl+o, 32g+jf] = (jl==jf) * (1 if o==0 else noise[64c+32g+jf])
    #   b4bp[2jl+o, g] = b4b[64c+32g+jl, o]
    aux3 = nc.dram_tensor("aux3", [64, 98], F32, kind="ExternalInput").ap()
    out = nc.dram_tensor("out", [1, J], F32, kind="ExternalOutput").ap()

    # stage-1 free-dim chunk plan: small first chunk (fast rampup), 1 MiB rest
    CHUNK_COLS = [1024, 3072, 4096, 4096, 4096]

    with tile.TileContext(nc) as tc:
        with (
            tc.tile_pool(name="stream", bufs=24) as sp,
            tc.tile_pool(name="res", bufs=1) as rp,
            tc.tile_pool(name="psum", bufs=1, space=bass.MemorySpace.PSUM) as pp,
        ):
            # small resident tensors on the scalar (ACT) HWDGE ring first
            xbf_t = rp.tile([128, 4], BF16)
            nc.scalar.dma_start(xbf_t[:, :], xbf)
            aux1_t = rp.tile([128, 256], F32)
            aux3_t = rp.tile([64, 98], F32)
            w5s = rp.tile([128, J * 4], BF16)

            def aux_dma():
                nc.scalar.dma_start(aux1_t[:, :], aux1)
                nc.scalar.dma_start(aux3_t[:, :], aux3)
                nc.scalar.dma_start(w5s[:, :], w5t)

            corrt = aux1_t[:, 0:128]
            bias4t = aux1_t[:, 128:256]
            dmask = aux3_t[:, 0:32]
            msk2 = aux3_t[:, 32:96]
            b4bp = aux3_t[:, 96:98]

            w2s = rp.tile([128, J * 2 * 64], BF16)
            w4s = rp.tile([128, (J // 2) * H], BF16)

            y1_a = pp.tile([128, 64], F32)
            y1_b = pp.tile([128, 64], F32)
            y1h = [y1_a, y1_b]
            zsrc = rp.tile([128, 128], BF16)
            nc.vector.memset(zsrc[:, :], 0.0)
            h1f = rp.tile([128, 128], F32)
            h1sb = rp.tile([128, 128], BF16)
            psum2 = pp.tile([128, J], F32)
            nc.vector.memset(psum2[:, :], 0.0)
            r1cols = rp.tile([128, J], BF16)
            psum4 = pp.tile([128, 128], F32)
            h4f = rp.tile([128, 128], F32)
            h4cols = rp.tile([128, 128], BF16)
            psum5g = pp.tile([64, J], F32)
            ttr_dump = rp.tile([64, J], F32)
            predcol = rp.tile([64, 2], F32)

            def s1_groups(chunk_cols, col0, half):
                """stage-1 matmuls for w1t cols [col0, ...); dual-ring stream.
                Consumed ic-outer (chunk-arrival order); y1 bank pre-cleared by
                a zero matmul so every pass accumulates with start=False."""
                nc.tensor.matmul(
                    y1h[half][:, 0:64], zsrc[:, 0:128], zsrc[:, 0:64],
                    start=True, stop=False, skip_group_check=True,
                )
                for gidx, ncols in enumerate(chunk_cols):
                    tiles_u = []
                    for ic in range(4):
                        wt = sp.tile([128, ncols], BF16, tag="w1s")
                        eng = nc.sync if ic % 2 == 0 else nc.scalar
                        eng.dma_start(
                            wt[:, :],
                            w1t[128 * ic : 128 * (ic + 1), col0 : col0 + ncols],
                        )
                        tiles_u.append(wt)
                    for ic in range(4):
                        for tt in range(ncols // 128):
                            t = col0 // 128 + tt
                            nc.tensor.matmul(
                                y1h[half][:, t % 64 : t % 64 + 1],
                                tiles_u[ic][:, 128 * tt : 128 * (tt + 1)],
                                xbf_t[:, ic : ic + 1],
                                start=False,
                                stop=(ic == 3 and gidx == len(chunk_cols) - 1),
                                skip_group_check=True,
                            )
                    col0 += ncols

            def tail_weights_dma(half):
                lo = 4096 * half
                nc.sync.dma_start(w2s[:, lo : lo + 2048], w2t[:, lo : lo + 2048])
                nc.scalar.dma_start(
                    w2s[:, lo + 2048 : lo + 4096], w2t[:, lo + 2048 : lo + 4096]
                )
                nc.scalar.dma_start(w4s[:, lo : lo + 2048], w4t[:, lo : lo + 2048])
                nc.sync.dma_start(
                    w4s[:, lo + 2048 : lo + 4096], w4t[:, lo + 2048 : lo + 4096]
                )

            def tail_half(half):
                """stages 2/4/5 for branches [32*half, 32*half+32)"""
                lo = 64 * half  # h1 col offset
                jlo = 32 * half
                # h1 = relu(y1 - corr)
                nc.vector.tensor_sub(
                    h1f[:, lo : lo + 64], y1h[half][:, 0:64], corrt[:, lo : lo + 64]
                )
                nc.vector.tensor_scalar_max(
                    h1sb[:, lo : lo + 64], h1f[:, lo : lo + 64], 0.0
                )
                # stage 2
                for j in range(jlo, jlo + 32):
                    off = 64 * (j % 2)
                    for hc in range(2):
                        nc.tensor.matmul(
                            psum2[off : off + 64, j : j + 1],
                            w2s[:, j * 128 + hc * 64 : j * 128 + hc * 64 + 64],
                            h1sb[:, 2 * j + hc : 2 * j + hc + 1],
                            start=(hc == 0),
                            stop=(hc == 1),
                        )
                nc.vector.tensor_scalar_max(
                    r1cols[:, jlo : jlo + 32], psum2[:, jlo : jlo + 32], 0.0
                )
                # stage 4 (pair-shared K=128 stationaries)
                for u_ in range(16 * half, 16 * half + 16):
                    for hc in range(2):
                        for par in range(2):
                            j = 2 * u_ + par
                            nc.tensor.matmul(
                                psum4[:, hc * 64 + j : hc * 64 + j + 1],
                                w4s[:, u_ * 256 + hc * 128 : u_ * 256 + hc * 128 + 128],
                                r1cols[:, j : j + 1],
                                start=True,
                                stop=True,
                            )
                for hc in range(2):
                    c = hc * 64 + jlo
                    nc.vector.tensor_add(
                        h4f[:, c : c + 32], psum4[:, c : c + 32], bias4t[:, c : c + 32]
                    )
                    nc.vector.tensor_scalar_max(
                        h4cols[:, c : c + 32], h4f[:, c : c + 32], 0.0
                    )
                # stage 5 (32-branch packed stationary, F=32)
                grp = half
                for hc in range(2):
                    nc.tensor.matmul(
                        psum5g[0:64, 32 * grp : 32 * grp + 32],
                        w5s[:, 128 * grp + 64 * hc : 128 * grp + 64 * hc + 64],
                        h4cols[:, 64 * hc + 32 * grp : 64 * hc + 32 * grp + 32],
                        start=(hc == 0),
                        stop=(hc == 1),
                    )
                nc.vector.tensor_mul(
                    ttr_dump[:, 32 * grp : 32 * grp + 32],
                    psum5g[0:64, 32 * grp : 32 * grp + 32],
                    dmask,
                )
                nc.vector.tensor_reduce(
                    predcol[:, grp : grp + 1],
                    ttr_dump[:, 32 * grp : 32 * grp + 32],
                    mybir.AxisListType.X,
                    mybir.AluOpType.add,
                )

            # ---- schedule: half A's tail stages run while half B streams ----
            s1_groups([2048] * 4, 0, 0)
            aux_dma()
            tail_weights_dma(0)
            tail_half(0)
            s1_groups([2048] * 4, 8192, 1)
            tail_weights_dma(1)
            tail_half(1)

            # pred = relu(predcol + b4bp); out = msk2-combine (noise folded in)
            nc.vector.tensor_add(predcol[:, :], predcol[:, :], b4bp)
            nc.vector.tensor_scalar_max(predcol[:, :], predcol[:, :], 0.0)
            psum6 = pp.tile([1, J], F32)
            for grp in range(2):
                nc.tensor.matmul(
                    psum6[0:1, 32 * grp : 32 * grp + 32],
                    predcol[:, grp : grp + 1],
                    msk2[:, 32 * grp : 32 * grp + 32],
                    start=True,
                    stop=True,
                )
            yrow = rp.tile([1, J], F32)
            nc.vector.tensor_copy(yrow[0:1, :], psum6[0:1, :])
            nc.sync.dma_start(out, yrow[0:1, :])

    nc.compile()
    _CACHE["nc"] = nc
    return nc


def _bf(a):
    return np.ascontiguousarray(a.astype(NP_BF16))


def prep_core_inputs(inputs, c):
    """Host-side shard + layout prep for core c. inputs are np float32 arrays."""
    x = inputs["x"][0]  # [512]
    gi = np.arange(J * c, J * (c + 1))
    xg = x[gi]
    jj = np.arange(J)

    W1a_c = inputs["W1a"][gi]  # [64, 256, 512]
    w1t = _bf(W1a_c.transpose(2, 0, 1).reshape(512, J * H))

    # self-excite path entirely on host (tiny), folded into stage-4 bias
    dW2 = inputs["W2a"][gi, :, gi]  # [64, 2]
    h2 = np.maximum(dW2 * xg[:, None] + inputs["b2a"][gi], 0.0)
    r2 = np.maximum((inputs["W2b"][gi, 0] * h2).sum(-1) + inputs["b2b"][gi, 0], 0.0)

    # correction computed from the bf16-rounded operands (exact cancellation
    # of the diagonal term the device's bf16 matmul actually added)
    dW1 = W1a_c[jj, :, gi].astype(NP_BF16).astype(np.float32)  # [64, 256]
    xg_bf = xg.astype(NP_BF16).astype(np.float32)
    corr_jh = dW1 * xg_bf[:, None]
    corrt = corr_jh.reshape(J, 2, 128).transpose(2, 0, 1).reshape(128, 128)

    W4a_c = inputs["W4a"][gi]  # [64, 256, 65]
    bias4_jh = inputs["b4a"][gi] + W4a_c[:, :, 64] * r2[:, None]
    # h4 col layout = hc*64 + j
    bias4t = bias4_jh.reshape(J, 2, 128).transpose(2, 1, 0).reshape(128, 128)

    aux1 = np.ascontiguousarray(
        np.concatenate([corrt, bias4t], axis=1), dtype=np.float32
    )
    xbf = _bf(x.reshape(4, 128).T)  # [128, 4]

    # aux3: dmask | msk2 | b4bp   (stage-5 diag-extract + final combine)
    noise_c = inputs["noise"][gi]
    dmask = np.repeat(np.eye(32, dtype=np.float32), 2, axis=0)  # [64, 32]
    msk2 = np.zeros((64, 64), np.float32)
    for grp in range(2):
        jf = np.arange(32)
        msk2[2 * jf, 32 * grp + jf] = 1.0
        msk2[2 * jf + 1, 32 * grp + jf] = noise_c[32 * grp + jf]
    b4bp = (
        inputs["b4b"][gi].reshape(2, 32, 2).transpose(1, 2, 0).reshape(64, 2)
    )  # [2jl+o, grp]
    aux3 = np.ascontiguousarray(
        np.concatenate([dmask, msk2, b4bp], axis=1), dtype=np.float32
    )

    W1b_c = inputs["W1b"][gi]  # [64, 64, 256]
    w2t = _bf(
        W1b_c.reshape(J, 64, 2, 128).transpose(3, 0, 2, 1).reshape(128, J * 2 * 64)
    )

    # stage 4: branch pairs stacked on partitions (even j -> rows 0..63)
    W4m = W4a_c[:, :, 0:64]  # [j, h, m]
    T4 = W4m.reshape(J // 2, 2, H, 64)  # [u, par, h, m]
    w4t = _bf(T4.transpose(1, 3, 0, 2).reshape(128, (J // 2) * H))

    # w5p[hl, grp*128 + hc*64 + 2jl+o] = W4b[g(32grp+jl), o, hc*128+hl]
    W4b_c = inputs["W4b"][gi]  # [64, 2, 256]
    w5t = _bf(
        W4b_c.reshape(2, 32, 2, 2, 128).transpose(4, 0, 3, 1, 2).reshape(128, J * 4)
    )

    return {
        "w1t": w1t, "w2t": w2t, "w4t": w4t, "w5t": w5t,
        "xbf": xbf, "aux1": aux1, "aux3": aux3,
    }


def run(inputs, trace=False, **kw):
    inputs = {k: np.asarray(v, dtype=np.float32) for k, v in inputs.items()}
    nc = _build_nc()
    in_maps = [prep_core_inputs(inputs, c) for c in range(NCORES)]
    res = run_bass_kernel_spmd(
        nc, in_maps, core_ids=list(range(NCORES)), trace=trace, **kw
    )
    out = np.concatenate([res.results[c]["out"] for c in range(NCORES)], axis=1)
    return out.astype(np.float32), res


def kernel(**inputs):
    out, _ = run(inputs)
    return out


# revision 29
# speedup vs baseline: 1.1088x; 1.0079x over previous
"""Trainium2 Bass kernel for CausalTrajectoryPrediction (batched per-branch MLPs).

Math (per branch n of N=512, H=256, M=64):
    x_off = x with element n zeroed ; x_diag = only element n kept
    h1 = relu(W1a[n] @ x_off)            # [H]
    r1 = relu(W1b[n] @ h1)               # [M]
    r2 = relu(W2b[n] @ relu(W2a[n] @ x_diag + b2a[n]) + b2b[n])   # [1]
    h4 = relu(W4a[n] @ [r1; r2] + b4a[n])                          # [H]
    pred = relu(W4b[n] @ h4 + b4b[n])                              # [2]
    out[n] = pred[0] + noise[n] * pred[1]

Device strategy (8 cores, 64 branches each, expert-parallel):
  * W1a[n] @ x_off == W1a[n] @ x - W1a[n,:,n]*x[n]; the shared matvec is done
    on-device (weights stationary, x moving), the diagonal correction is a
    tiny host-side gather shipped as a [128,128] f32 tile (computed from the
    bf16-rounded operands so the subtraction cancels exactly).
  * The self-excite path (r2) only touches W2a's diagonal -> computed on host
    (512 branches x ~8 flops) and folded into an effective stage-4 bias:
    bias4_eff = b4a + W4a[:,:,64] * r2.  Stage 4 then contracts over m=0..63.
  * Weights + moving activations in fp16 (PSUM accumulation f32, all bias /
    correction math f32): f32 matmuls lower to 2x LDWEIGHTS+MATMUL passes on
    TRN2, while fp16 stationaries are single-pass with fast-weight-load and
    halve the 32MB weight stream; fp16's 10 mantissa bits keep l2 rel err
    ~6e-4 (bf16 was ~5e-3).
  * All weights are pre-transposed on host into [K-partition, free] layouts so
    each per-branch matvec is a single matmul with the activation vector as
    the moving operand; outputs land on PSUM partitions in exactly the layout
    the next stage consumes.
  * The W1a stream is split across BOTH HWDGE rings (sync + scalar, each ring
    FIFO) - dual rings sustain ~410 GB/s vs ~330 single-ring.  Stage-1 psum
    banks are pre-cleared by a zero matmul so the 4 i-chunk passes accumulate
    with start=False in chunk-ARRIVAL order (no intra-group ordering stalls).
  * Branches are processed in two halves: half A's tail stages (2/4/5) run on
    the PE while half B's weights stream, hiding most of the tail latency.
"""

import ml_dtypes
import numpy as np

import concourse.bacc as bacc
import concourse.bass as bass
import concourse.mybir as mybir
import concourse.tile as tile
from concourse.bass_utils import run_bass_kernel_spmd

F32 = mybir.dt.float32
BF16 = mybir.dt.float16
NP_BF16 = np.float16
NCORES = 8
N, H, M = 512, 256, 64
J = N // NCORES  # 64 branches per core

_CACHE = {}


def _build_nc():
    if "nc" in _CACHE:
        return _CACHE["nc"]

    nc = bacc.Bacc(
        "TRN2", target_bir_lowering=False, debug=False, enable_asserts=False,
        num_devices=NCORES,
    )

    # --- DRAM I/O (per-core shapes) ---
    # w1t[i, j*256+h]              = W1a[g, h, i]           (g = 64*core + j)
    # w2t[hl, j*128+hc*64+m]       = W1b[g, m, hc*128+hl]
    # w4t[64*(j%2)+m, (j//2)*256+h]= W4a[g, h, m]   (m<64; col 64 folded in bias)
    # w5t[hl, j*4+hc*2+o]          = W4b[g, o, hc*128+hl]
    # xbf[p, ic] = x[128*ic+p]                     (bf16 moving operand)
    # aux1 = [corr(128) | bias4(128)]   -> [128, 256] f32
    #   corr[p, 2j+hh]  = bf16(W1a[g, hh*128+p, g]) * bf16(x[g])
    #   bias4[p, 2j+hh] = b4a[g, hh*128+p] + W4a[g, hh*128+p, 64]*r2_host[g]
    # aux2 = [b4bt(64) | noise2(64) | ones(1)]     -> [2, 129] f32
    w1t = nc.dram_tensor("w1t", [512, J * H], BF16, kind="ExternalInput").ap()
    w2t = nc.dram_tensor("w2t", [128, J * 2 * 64], BF16, kind="ExternalInput").ap()
    w4t = nc.dram_tensor("w4t", [128, (J // 2) * H], BF16, kind="ExternalInput").ap()
    w5t = nc.dram_tensor("w5t", [128, J * 4], BF16, kind="ExternalInput").ap()
    xbf = nc.dram_tensor("xbf", [128, 4], BF16, kind="ExternalInput").ap()
    aux1 = nc.dram_tensor("aux1", [128, 256], F32, kind="ExternalInput").ap()
    # aux3 = [dmask(32) | msk2(64) | b4bp(2)] -> [64, 98] f32
    #   dmask[2jl+o, jf] = (jl==jf)
    #   msk2[2jl+o, 32g+jf] = (jl==jf) * (1 if o==0 else noise[64c+32g+jf])
    #   b4bp[2jl+o, g] = b4b[64c+32g+jl, o]
    aux3 = nc.dram_tensor("aux3", [64, 98], F32, kind="ExternalInput").ap()
    out = nc.dram_tensor("out", [1, J], F32, kind="ExternalOutput").ap()

    # stage-1 free-dim chunk plan: small first chunk (fast rampup), 1 MiB rest
    CHUNK_COLS = [1024, 3072, 4096, 4096, 4096]

    with tile.TileContext(nc) as tc:
        with (
            tc.tile_pool(name="stream", bufs=12) as sp,
            tc.tile_pool(name="res", bufs=1) as rp,
            tc.tile_pool(name="psum", bufs=1, space=bass.MemorySpace.PSUM) as pp,
        ):
            # small resident tensors on the scalar (ACT) HWDGE ring first
            xbf_t = rp.tile([128, 4], BF16)
            nc.scalar.dma_start(xbf_t[:, :], xbf)
            aux1_t = rp.tile([128, 256], F32)
            aux3_t = rp.tile([64, 98], F32)
            w5s = rp.tile([128, J * 4], BF16)

            def aux_dma():
                nc.scalar.dma_start(aux1_t[:, :], aux1)
                nc.scalar.dma_start(aux3_t[:, :], aux3)
                nc.scalar.dma_start(w5s[:, :], w5t)

            corrt = aux1_t[:, 0:128]
            bias4t = aux1_t[:, 128:256]
            dmask = aux3_t[:, 0:32]
            msk2 = aux3_t[:, 32:96]
            b4bp = aux3_t[:, 96:98]

            w2s = rp.tile([128, J * 2 * 64], BF16)
            w4s = rp.tile([128, (J // 2) * H], BF16)

            y1_a = pp.tile([128, 64], F32)
            y1_b = pp.tile([128, 64], F32)
            y1h = [y1_a, y1_b]
            zsrc = rp.tile([128, 128], BF16)
            nc.vector.memset(zsrc[:, :], 0.0)
            h1f = rp.tile([128, 128], F32)
            h1sb = rp.tile([128, 128], BF16)
            psum2 = pp.tile([128, J], F32)
            nc.vector.memset(psum2[:, :], 0.0)
            r1cols = rp.tile([128, J], BF16)
            psum4 = pp.tile([128, 128], F32)
            h4f = rp.tile([128, 128], F32)
            h4cols = rp.tile([128, 128], BF16)
            psum5g = pp.tile([64, J], F32)
            ttr_dump = rp.tile([64, J], F32)
            predcol = rp.tile([64, 2], F32)

            def s1_groups(chunk_cols, col0, half, clear=True):
                """stage-1 matmuls for w1t cols [col0, ...); dual-ring stream.
                Consumed ic-outer (chunk-arrival order); y1 bank pre-cleared by
                a zero matmul so every pass accumulates with start=False."""
                if clear:
                    nc.tensor.matmul(
                        y1h[half][:, 0:64], zsrc[:, 0:128], zsrc[:, 0:64],
                        start=True, stop=False, skip_group_check=True,
                    )
                w1t_r = w1t.rearrange("(ic p) c -> p ic c", p=128)
                for gidx, ncols in enumerate(chunk_cols):
                    wlo = sp.tile([128, 2, ncols], BF16, tag="w1s")
                    whi = sp.tile([128, 2, ncols], BF16, tag="w1s")
                    nc.sync.dma_start(wlo[:, :, :], w1t_r[:, 0:2, col0 : col0 + ncols])
                    nc.scalar.dma_start(whi[:, :, :], w1t_r[:, 2:4, col0 : col0 + ncols])
                    pair = [wlo, wlo, whi, whi]
                    for ic in range(4):
                        for tt in range(ncols // 128):
                            t = col0 // 128 + tt
                            nc.tensor.matmul(
                                y1h[half][:, t % 64 : t % 64 + 1],
                                pair[ic][:, ic % 2, 128 * tt : 128 * (tt + 1)],
                                xbf_t[:, ic : ic + 1],
                                start=False,
                                stop=(ic == 3 and gidx == len(chunk_cols) - 1),
                                skip_group_check=True,
                            )
                    col0 += ncols

            def tail_weights_dma(half):
                lo = 4096 * half
                nc.sync.dma_start(w2s[:, lo : lo + 2048], w2t[:, lo : lo + 2048])
                nc.scalar.dma_start(
                    w2s[:, lo + 2048 : lo + 4096], w2t[:, lo + 2048 : lo + 4096]
                )
                nc.scalar.dma_start(w4s[:, lo : lo + 2048], w4t[:, lo : lo + 2048])
                nc.sync.dma_start(
                    w4s[:, lo + 2048 : lo + 4096], w4t[:, lo + 2048 : lo + 4096]
                )

            def h1_relu(half):
                lo = 64 * half
                nc.vector.tensor_sub(
                    h1f[:, lo : lo + 64], y1h[half][:, 0:64], corrt[:, lo : lo + 64]
                )
                nc.vector.tensor_scalar_max(
                    h1sb[:, lo : lo + 64], h1f[:, lo : lo + 64], 0.0
                )

            def s2_half(half):
                jlo = 32 * half
                for j in range(jlo, jlo + 32):
                    off = 64 * (j % 2)
                    for hc in range(2):
                        nc.tensor.matmul(
                            psum2[off : off + 64, j : j + 1],
                            w2s[:, j * 128 + hc * 64 : j * 128 + hc * 64 + 64],
                            h1sb[:, 2 * j + hc : 2 * j + hc + 1],
                            start=(hc == 0),
                            stop=(hc == 1),
                        )
                nc.vector.tensor_scalar_max(
                    r1cols[:, jlo : jlo + 32], psum2[:, jlo : jlo + 32], 0.0
                )

            def s4_half(half):
                jlo = 32 * half
                for u_ in range(16 * half, 16 * half + 16):
                    for hc in range(2):
                        for par in range(2):
                            j = 2 * u_ + par
                            nc.tensor.matmul(
                                psum4[:, hc * 64 + j : hc * 64 + j + 1],
                                w4s[:, u_ * 256 + hc * 128 : u_ * 256 + hc * 128 + 128],
                                r1cols[:, j : j + 1],
                                start=True,
                                stop=True,
                            )
                for hc in range(2):
                    c = hc * 64 + jlo
                    nc.vector.tensor_add(
                        h4f[:, c : c + 32], psum4[:, c : c + 32], bias4t[:, c : c + 32]
                    )
                    nc.vector.tensor_scalar_max(
                        h4cols[:, c : c + 32], h4f[:, c : c + 32], 0.0
                    )

            def s5_half(half):
                grp = half
                for hc in range(2):
                    nc.tensor.matmul(
                        psum5g[0:64, 32 * grp : 32 * grp + 32],
                        w5s[:, 128 * grp + 64 * hc : 128 * grp + 64 * hc + 64],
                        h4cols[:, 64 * hc + 32 * grp : 64 * hc + 32 * grp + 32],
                        start=(hc == 0),
                        stop=(hc == 1),
                    )
                nc.vector.tensor_mul(
                    ttr_dump[:, 32 * grp : 32 * grp + 32],
                    psum5g[0:64, 32 * grp : 32 * grp + 32],
                    dmask,
                )
                nc.vector.tensor_reduce(
                    predcol[:, grp : grp + 1],
                    ttr_dump[:, 32 * grp : 32 * grp + 32],
                    mybir.AxisListType.X,
                    mybir.AluOpType.add,
                )

            # ---- schedule: half A's tail stages fill half B's stream gaps ----
            s1_groups([1024, 1024, 2048, 2048, 2048], 0, 0)
            aux_dma()
            tail_weights_dma(0)
            h1_relu(0)
            s1_groups([2048], 8192, 1)
            s2_half(0)
            s1_groups([2048], 10240, 1, clear=False)
            s4_half(0)
            s1_groups([2048], 12288, 1, clear=False)
            s5_half(0)
            s1_groups([2048], 14336, 1, clear=False)
            tail_weights_dma(1)
            h1_relu(1)
            s2_half(1)
            s4_half(1)
            s5_half(1)

            # pred = relu(predcol + b4bp); out = msk2-combine (noise folded in)
            nc.vector.tensor_add(predcol[:, :], predcol[:, :], b4bp)
            nc.vector.tensor_scalar_max(predcol[:, :], predcol[:, :], 0.0)
            psum6 = pp.tile([1, J], F32)
            for grp in range(2):
                nc.tensor.matmul(
                    psum6[0:1, 32 * grp : 32 * grp + 32],
                    predcol[:, grp : grp + 1],
                    msk2[:, 32 * grp : 32 * grp + 32],
                    start=True,
                    stop=True,
                )
            yrow = rp.tile([1, J], F32)
            nc.vector.tensor_copy(yrow[0:1, :], psum6[0:1, :])
            nc.sync.dma_start(out, yrow[0:1, :])

    nc.compile()
    _CACHE["nc"] = nc
    return nc


def _bf(a):
    return np.ascontiguousarray(a.astype(NP_BF16))


def prep_core_inputs(inputs, c):
    """Host-side shard + layout prep for core c. inputs are np float32 arrays."""
    x = inputs["x"][0]  # [512]
    gi = np.arange(J * c, J * (c + 1))
    xg = x[gi]
    jj = np.arange(J)

    W1a_c = inputs["W1a"][gi]  # [64, 256, 512]
    w1t = _bf(W1a_c.transpose(2, 0, 1).reshape(512, J * H))

    # self-excite path entirely on host (tiny), folded into stage-4 bias
    dW2 = inputs["W2a"][gi, :, gi]  # [64, 2]
    h2 = np.maximum(dW2 * xg[:, None] + inputs["b2a"][gi], 0.0)
    r2 = np.maximum((inputs["W2b"][gi, 0] * h2).sum(-1) + inputs["b2b"][gi, 0], 0.0)

    # correction computed from the bf16-rounded operands (exact cancellation
    # of the diagonal term the device's bf16 matmul actually added)
    dW1 = W1a_c[jj, :, gi].astype(NP_BF16).astype(np.float32)  # [64, 256]
    xg_bf = xg.astype(NP_BF16).astype(np.float32)
    corr_jh = dW1 * xg_bf[:, None]
    corrt = corr_jh.reshape(J, 2, 128).transpose(2, 0, 1).reshape(128, 128)

    W4a_c = inputs["W4a"][gi]  # [64, 256, 65]
    bias4_jh = inputs["b4a"][gi] + W4a_c[:, :, 64] * r2[:, None]
    # h4 col layout = hc*64 + j
    bias4t = bias4_jh.reshape(J, 2, 128).transpose(2, 1, 0).reshape(128, 128)

    aux1 = np.ascontiguousarray(
        np.concatenate([corrt, bias4t], axis=1), dtype=np.float32
    )
    xbf = _bf(x.reshape(4, 128).T)  # [128, 4]

    # aux3: dmask | msk2 | b4bp   (stage-5 diag-extract + final combine)
    noise_c = inputs["noise"][gi]
    dmask = np.repeat(np.eye(32, dtype=np.float32), 2, axis=0)  # [64, 32]
    msk2 = np.zeros((64, 64), np.float32)
    for grp in range(2):
        jf = np.arange(32)
        msk2[2 * jf, 32 * grp + jf] = 1.0
        msk2[2 * jf + 1, 32 * grp + jf] = noise_c[32 * grp + jf]
    b4bp = (
        inputs["b4b"][gi].reshape(2, 32, 2).transpose(1, 2, 0).reshape(64, 2)
    )  # [2jl+o, grp]
    aux3 = np.ascontiguousarray(
        np.concatenate([dmask, msk2, b4bp], axis=1), dtype=np.float32
    )

    W1b_c = inputs["W1b"][gi]  # [64, 64, 256]
    w2t = _bf(
        W1b_c.reshape(J, 64, 2, 128).transpose(3, 0, 2, 1).reshape(128, J * 2 * 64)
    )

    # stage 4: branch pairs stacked on partitions (even j -> rows 0..63)
    W4m = W4a_c[:, :, 0:64]  # [j, h, m]
    T4 = W4m.reshape(J // 2, 2, H, 64)  # [u, par, h, m]
    w4t = _bf(T4.transpose(1, 3, 0, 2).reshape(128, (J // 2) * H))

    # w5p[hl, grp*128 + hc*64 + 2jl+o] = W4b[g(32grp+jl), o, hc*128+hl]
    W4b_c = inputs["W4b"][gi]  # [64, 2, 256]
    w5t = _bf(
        W4b_c.reshape(2, 32, 2, 2, 128).transpose(4, 0, 3, 1, 2).reshape(128, J * 4)
    )

    return {
        "w1t": w1t, "w2t": w2t, "w4t": w4t, "w5t": w5t,
        "xbf": xbf, "aux1": aux1, "aux3": aux3,
    }


def run(inputs, trace=False, **kw):
    inputs = {k: np.asarray(v, dtype=np.float32) for k, v in inputs.items()}
    nc = _build_nc()
    in_maps = [prep_core_inputs(inputs, c) for c in range(NCORES)]
    res = run_bass_kernel_spmd(
        nc, in_maps, core_ids=list(range(NCORES)), trace=trace, **kw
    )
    out = np.concatenate([res.results[c]["out"] for c in range(NCORES)], axis=1)
    return out.astype(np.float32), res


def kernel(**inputs):
    out, _ = run(inputs)
    return out
